# revision 9
# baseline (speedup 1.0000x reference)
"""Cross-attention head kernel for Trainium2 (8 NeuronCores, batch-parallel).

Computes, per batch element b:
    Q = query[b] @ Wq + bq          [T, H]
    K = kv[b]    @ Wk + bk          [S, H]
    V = kv[b]    @ Wv + bv          [S, H]
    scores = Q @ K.T / sqrt(H) + maskbias[b]   (maskbias = -1e9 where mask==0)
    out[b] = softmax(scores) @ V

Sharding: one batch element per NeuronCore (B == n_cores == 8).

Device-side layout (per core):
  - Host pre-transposes query/key_value to [D, T] (and casts to fp16) so
    the d_model contraction has d on SBUF partitions, as the PE needs.
  - Projections produce Q^T [H, T] and (K^T; V^T) packed [2H, S] in PSUM
    (fp32), copied to SBUF as fp16.
  - Scores are computed transposed: S^T[s, t] tiles [128, t] so that the
    mask (per-s) is the per-partition bias of the ACT exp, and exp(S^T)
    tiles feed the PV matmul directly as the moving operand.
  - PV matmul uses lhsT = [V | ones]; row H of the fp32 accumulator then
    holds the softmax denominators for free.
  - The [H+1, t] accumulator is PE-transposed back to [t, H+1] (fp32); a
    DVE reciprocal + tensor_scalar multiply normalizes; output rows DMA
    out contiguously.

All matmul operands are fp16 (1 cycle/row on the PE, like bf16, but with
a 10-bit mantissa: measured end-to-end rel-l2 error ~6e-4). PSUM
accumulation is fp32 throughout; the normalize path is fp32.
"""

import sys

sys.path.insert(0, "/opt/trn_rl_repo")

from contextlib import ExitStack

import numpy as np

import concourse.bass as bass
import concourse.tile as tile
from concourse import bacc, bass_utils, mybir
from concourse.masks import make_identity

B, T, S, D, H = 8, 2048, 2048, 1024, 64
F32 = mybir.dt.float32
F16 = mybir.dt.float16
NEG = -1.0e9  # additive mask value; exp(scale*NEG) underflows to 0 in f32

DC = D // 128  # 8 d_model chunks
SC = S // 128  # 16 src-seq chunks
NT = 512  # matmul moving-dim tile (one fp32 PSUM bank per output tile)

Act = mybir.ActivationFunctionType


def _build(tc, out, qT, kvT, mb, wq, wkv, bq, bkv):
    nc = tc.nc
    with ExitStack() as ctx:
        consts = ctx.enter_context(tc.tile_pool(name="consts", bufs=1))
        proj_in = ctx.enter_context(tc.tile_pool(name="proj_in", bufs=DC))
        proj_sb = ctx.enter_context(tc.tile_pool(name="proj_sb", bufs=1))

        # ---- constants -------------------------------------------------
        wq_sb = consts.tile([128, DC, H], F16)  # wq_sb[p, c, h] = Wq[c*128+p, h]
        nc.sync.dma_start(out=wq_sb, in_=wq.rearrange("(c p) h -> p c h", p=128))
        # wkv is host-packed [Wk | Wv]: a single producer DMA per tile —
        # an Ldweights can carry only one sync wait.
        wkv_sb = consts.tile([128, DC, 2 * H], F16)
        nc.sync.dma_start(out=wkv_sb, in_=wkv.rearrange("(c p) h -> p c h", p=128))
        bq_sb = consts.tile([H, 1], F32)
        nc.sync.dma_start(out=bq_sb, in_=bq.rearrange("(h o) -> h o", o=1))
        bkv_sb = consts.tile([2 * H, 1], F32)
        nc.sync.dma_start(out=bkv_sb, in_=bkv.rearrange("(h o) -> h o", o=1))
        mb_sb = consts.tile([128, SC], F32)  # mask bias, chunk j in column j
        nc.sync.dma_start(out=mb_sb, in_=mb.rearrange("(c p) -> p c", p=128))
        ident16 = consts.tile([128, 128], F16)
        make_identity(nc, ident16)
        ident32 = consts.tile([128, 128], F32)
        make_identity(nc, ident32)

        # ---- phase 1: projections -------------------------------------
        qT_sb = proj_sb.tile([H, T], F16)
        kvT_sb = proj_sb.tile([2 * H, S], F16)
        psum_pr_cm = tc.tile_pool(name="psum_pr", bufs=1, space="PSUM")
        psum_pr = psum_pr_cm.__enter__()
        # psum_q [H, T] accumulates Q^T; psum_kv [2H, S] = [K^T; V^T]
        psum_q = psum_pr.tile([H, T], F32)
        psum_kv = psum_pr.tile([2 * H, S], F32)
        for c in range(DC):
            qc = proj_in.tile([128, T], F16, tag="qc")
            nc.sync.dma_start(out=qc, in_=qT[c * 128 : (c + 1) * 128, :])
            kvc = proj_in.tile([128, S], F16, tag="kvc")
            nc.sync.dma_start(out=kvc, in_=kvT[c * 128 : (c + 1) * 128, :])
            for n in range(T // NT):
                sl = bass.ts(n, NT)
                nc.tensor.matmul(
                    psum_q[:, sl],
                    wq_sb[:, c, :],
                    qc[:, sl],
                    start=(c == 0),
                    stop=(c == DC - 1),
                )
                nc.tensor.matmul(
                    psum_kv[:, sl],
                    wkv_sb[:, c, :],
                    kvc[:, sl],
                    start=(c == 0),
                    stop=(c == DC - 1),
                )

        # move projections to SBUF (+ bias), casting to fp16
        nc.scalar.activation(qT_sb, psum_q, Act.Identity, bias=bq_sb)
        nc.scalar.activation(kvT_sb, psum_kv, Act.Identity, bias=bkv_sb)
        psum_pr_cm.__exit__(None, None, None)

        # V [s, h] tiles with a ones column appended: v_sb[:, j, :] = [V_j | 1]
        v_sb = proj_sb.tile([128, SC, H + 1], F16)
        nc.vector.memset(v_sb[:, :, H : H + 1], 1.0)
        with tc.tile_pool(name="psum_vt", bufs=2, space="PSUM") as psum_vt:
            for j in range(SC):
                pv = psum_vt.tile([128, H], F16)
                # transpose V^T[:, j*128:...] -> V chunk [128, H].
                # ident16[H:2H, H:2H] is an identity block at base partition
                # H, matching the base partition of the V^T rows.
                nc.tensor.transpose(
                    pv,
                    kvT_sb[H : 2 * H, j * 128 : (j + 1) * 128],
                    ident16[H : 2 * H, H : 2 * H],
                )
                nc.vector.tensor_copy(v_sb[:, j, 0:H], pv)

        # ---- phase 2: attention ---------------------------------------
        attn_ps = ctx.enter_context(tc.tile_pool(name="attn_ps", bufs=2, space="PSUM"))
        acc_ps = ctx.enter_context(tc.tile_pool(name="acc_ps", bufs=1, space="PSUM"))
        tr_ps = ctx.enter_context(tc.tile_pool(name="tr_ps", bufs=1, space="PSUM"))
        es_pool = ctx.enter_context(tc.tile_pool(name="es", bufs=6))
        fin = ctx.enter_context(tc.tile_pool(name="fin", bufs=3))

        TH = 1024  # t-half width
        for th in range(T // TH):
            acc = acc_ps.tile([H + 1, TH], F32)  # [out^T ; denom] accumulator
            for j in range(SC):
                ps = attn_ps.tile([128, TH], F32)
                for n in range(TH // NT):
                    nc.tensor.matmul(
                        ps[:, bass.ts(n, NT)],
                        kvT_sb[0:H, j * 128 : (j + 1) * 128],
                        qT_sb[:, th * TH + n * NT : th * TH + (n + 1) * NT],
                        start=True,
                        stop=True,
                    )
                es = es_pool.tile([128, TH], F16)
                nc.scalar.activation(
                    es, ps, Act.Exp, bias=mb_sb[:, j : j + 1], scale=0.125
                )
                for n in range(TH // NT):
                    nc.tensor.matmul(
                        acc[:, bass.ts(n, NT)],
                        v_sb[:, j, :],
                        es[:, bass.ts(n, NT)],
                        start=(j == 0),
                        stop=(j == SC - 1),
                    )

            # finalize this t-half: transpose back, normalize, store
            accT = fin.tile([H + 1, TH], F32, tag="accT")
            nc.scalar.copy(accT, acc)
            for q in range(TH // 128):
                pt = tr_ps.tile([128, H + 1], F32)
                nc.tensor.transpose(
                    pt, accT[:, q * 128 : (q + 1) * 128], ident32[0 : H + 1, 0 : H + 1]
                )
                rc = fin.tile([128, 1], F32, tag="rc")
                nc.vector.reciprocal(rc, pt[:, H : H + 1])
                ob = fin.tile([128, H], F32, tag="ob")
                nc.vector.tensor_scalar_mul(ob, pt[:, 0:H], rc)
                r0 = th * TH + q * 128
                nc.sync.dma_start(out=out[r0 : r0 + 128, :], in_=ob)


def build_kernel():
    nc = bacc.Bacc("TRN2", target_bir_lowering=False, debug=False)
    qT = nc.dram_tensor("qT", [D, T], F16, kind="ExternalInput").ap()
    kvT = nc.dram_tensor("kvT", [D, S], F16, kind="ExternalInput").ap()
    mb = nc.dram_tensor("mb", [S], F32, kind="ExternalInput").ap()
    wq = nc.dram_tensor("wq", [D, H], F16, kind="ExternalInput").ap()
    wkv = nc.dram_tensor("wkv", [D, 2 * H], F16, kind="ExternalInput").ap()
    bq = nc.dram_tensor("bq", [H], F32, kind="ExternalInput").ap()
    bkv = nc.dram_tensor("bkv", [2 * H], F32, kind="ExternalInput").ap()
    out = nc.dram_tensor("out", [T, H], F32, kind="ExternalOutput").ap()
    with tile.TileContext(nc) as tc:
        _build(tc, out, qT, kvT, mb, wq, wkv, bq, bkv)
    nc.compile()
    return nc


_NC_CACHE = {}


def _get_nc():
    if "nc" not in _NC_CACHE:
        _NC_CACHE["nc"] = build_kernel()
    return _NC_CACHE["nc"]


def kernel(query, key_value, mask, Wq, bq, Wk, bk, Wv, bv, **run_kwargs):
    query = np.asarray(query, dtype=np.float32)
    key_value = np.asarray(key_value, dtype=np.float32)
    qT = np.ascontiguousarray(query.transpose(0, 2, 1)).astype(np.float16)
    kvT = np.ascontiguousarray(key_value.transpose(0, 2, 1)).astype(np.float16)
    mb = np.where(np.asarray(mask) == 0, np.float32(NEG), np.float32(0.0)).astype(
        np.float32
    )
    w = {
        "wq": np.asarray(Wq).astype(np.float16),
        "wkv": np.concatenate(
            [np.asarray(Wk), np.asarray(Wv)], axis=1
        ).astype(np.float16),
        "bq": np.ascontiguousarray(np.asarray(bq, np.float32)),
        "bkv": np.concatenate(
            [np.asarray(bk), np.asarray(bv)]
        ).astype(np.float32),
    }
    in_maps = [
        {"qT": qT[b], "kvT": kvT[b], "mb": mb[b], **w} for b in range(B)
    ]
    nc = _get_nc()
    res = bass_utils.run_bass_kernel_spmd(
        nc, in_maps, core_ids=list(range(B)), **run_kwargs
    )
    out = np.stack([res.results[b]["out"] for b in range(B)], axis=0)
    if run_kwargs:
        return out, res
    return out
